# revision 31
# baseline (speedup 1.0000x reference)
"""CoAttentionFusion Trainium2 kernel.

Full-input contract: kernel(**inputs) takes the complete (unsharded) numpy
inputs and returns (out_p, out_s) matching the fp32 reference. Internally
shards batch 16 -> 2 per core across 8 NeuronCores (weights replicated),
builds one SPMD Bass program, and runs it via run_bass_kernel_spmd.

Math per batch b (L1=L2=512, D=512, H=8, HD=128):
  aff_h = tanh(P @ W_aff[h] @ S^T) * (pm_i * sm_j)
  pp = (P @ W_p) head-split; ps = (S @ W_s) head-split
  wp_h = relu(aff_h^T @ pp_h)  -> pool_p = max_h wp_h   (B, L2, HD)
  ws_h = relu(aff_h  @ ps_h)  -> pool_s = max_h ws_h   (B, L1, HD)
  out_p = relu([P, pool_s] @ W_fp + b_fp)
  out_s = relu([S, pool_p] @ W_fs + b_fs)

Layout trick: every matmul contracting P over D needs P^T (D on partitions),
and the final fused matmuls need [P^T; pool_s^T] as lhsT, so we compute the
weighted sums directly in transposed orientation:
  ws_h^T (d,i) = ps_h^T-as-lhsT @ A^T ; wp_h^T (d,j) = pp_h-as-lhsT @ A
which makes the head pools land exactly in the lhsT layout of the final
matmuls. Masks (>=0) commute with relu/max and are folded into pp/ps rows
and one elementwise multiply on each pool.

Precision (K_PREC): "f32" all-fp32; "r" fp32r everywhere (PE runs fp32r at
4x the fp32 rate for 512-wide moving operands); "rb" fp32r for the affinity
chain + projections + final matmuls, bf16 for the post-tanh tensors.
"""

import os

import numpy as np

import concourse.bacc as bacc
import concourse.mybir as mybir
import concourse.tile as tile
from concourse import bass_utils
from concourse.masks import make_identity

# Problem constants (hardcoded per contract).
B = 16
L = 512  # L1 == L2
D = 512
H = 8
INNER = 1024
HD = INNER // H  # 128
CONCAT = D + HD  # 640
P = 128
NT = L // P  # 4
NCORES = 8
BPC = B // NCORES  # batches per core

F32 = mybir.dt.float32
F32R = mybir.dt.float32r
BF16 = mybir.dt.bfloat16

PREC = os.environ.get("K_PREC", "b")


def _build_program(masks_trivial: bool, bias_trivial: bool, prec: str):
    if prec == "f32":
        dt1, dt2, dt_in = F32, F32, F32
    elif prec == "r":
        dt1, dt2, dt_in = F32R, F32R, F32
    elif prec == "rb":
        dt1, dt2, dt_in = F32R, BF16, F32
    elif prec == "b":
        dt1, dt2, dt_in = BF16, BF16, BF16
    else:
        raise ValueError(prec)

    nc = bacc.Bacc(
        "TRN2",
        target_bir_lowering=False,
        debug=False,
        enable_asserts=False,
        num_devices=NCORES,
    )

    def din(name, shape, dt=F32):
        return nc.dram_tensor(name, list(shape), dt, kind="ExternalInput").ap()

    def dout(name, shape):
        return nc.dram_tensor(name, list(shape), F32, kind="ExternalOutput").ap()

    primary = din("primary", (BPC, L, D), dt_in)
    secondary = din("secondary", (BPC, L, D), dt_in)
    pmask = din("primary_mask", (BPC, L))
    smask = din("secondary_mask", (BPC, L))
    w_aff = din("W_aff", (H, D, D), dt1)
    w_p = din("W_p", (D, INNER), dt1)
    w_s = din("W_s", (D, INNER), dt1)
    w_fp = din("W_fp", (CONCAT, D), dt1)
    b_fp = din("b_fp", (D,))
    w_fs = din("W_fs", (CONCAT, D), dt1)
    b_fs = din("b_fs", (D,))
    out_p = dout("out_p", (BPC, L, D))
    out_s = dout("out_s", (BPC, L, D))

    with tile.TileContext(nc) as tc:
        _body(
            tc,
            primary, secondary, pmask, smask,
            w_aff, w_p, w_s, w_fp, b_fp, w_fs, b_fs,
            out_p, out_s,
            masks_trivial, bias_trivial, dt1, dt2, dt_in,
        )
    nc.compile()
    return nc


def _body(
    tc,
    primary, secondary, pmask, smask,
    w_aff, w_p, w_s, w_fp, b_fp, w_fs, b_fs,
    out_p, out_s,
    masks_trivial, bias_trivial, dt1, dt2, dt_in,
):
    nc = tc.nc
    TANH = mybir.ActivationFunctionType.Tanh
    RELU = mybir.ActivationFunctionType.Relu
    hbufs = 2 if dt2 == BF16 else 1

    with (
        tc.tile_pool(name="consts", bufs=1) as consts,
        tc.tile_pool(name="wpool", bufs=1) as wpool,
        tc.tile_pool(name="waffp", bufs=2) as waffp,
        tc.tile_pool(name="bpool", bufs=2 if dt1 == BF16 else 1) as bpool,
        tc.tile_pool(name="hpool", bufs=hbufs) as hpool,
        tc.tile_pool(name="iopool", bufs=2) as iopool,
        tc.tile_pool(name="adram", bufs=3, space="DRAM") as adram,
        tc.tile_pool(name="psum", bufs=2, space="PSUM") as psum,
    ):
        MMB = 4  # psum bufs for matmul accumulation groups

        idents = {}

        def ident_for(dt):
            if dt not in idents:
                name = f"ident_{dt.name}"
                t = consts.tile([P, P], dt, name=name, tag=name)
                if dt == F32R:
                    fi = ident_for(F32)
                    nc.vector.tensor_copy(out=t[:], in_=fi[:])
                else:
                    make_identity(nc, t)
                idents[dt] = t
            return idents[dt]

        xbar_io = dt1 == dt_in and mybir.dt.size(dt_in) == 2

        # pt/st per batch, memoized so batch 0's can be issued before the
        # weight burst (the XBAR transpose is a single shared resource and
        # crawls if it overlaps the 8.5 MB weight stream on the HBM).
        ptst = {}

        def make_ptst(b):
            if b not in ptst:
                pt = bpool.tile([P, NT, L], dt1, name="pt", tag="pt")
                st = bpool.tile([P, NT, L], dt1, name="st", tag="st")
                if xbar_io:
                    # pt[p, eo, i] = primary[b][i, eo*128+p]
                    nc.sync.dma_start_transpose(pt[:], primary[b])
                    nc.sync.dma_start_transpose(st[:], secondary[b])
                else:
                    p_nat = iopool.tile([P, NT, D], dt_in, name="p_nat", tag="p_nat")
                    nc.sync.dma_start(
                        p_nat[:], primary[b].rearrange("(io p) e -> p io e", p=P)
                    )
                    s_nat = iopool.tile([P, NT, D], dt_in, name="s_nat", tag="s_nat")
                    nc.sync.dma_start(
                        s_nat[:], secondary[b].rearrange("(jo p) e -> p jo e", p=P)
                    )
                    transpose_512(pt, p_nat, ident_for(dt_in), dt_in)
                    transpose_512(st, s_nat, ident_for(dt_in), dt_in)
                ptst[b] = (pt, st)
            return ptst[b]

        if xbar_io:
            make_ptst(0)

        # ---- resident weights -------------------------------------------
        # Weights ride the gpsimd sequencer (its DIRECT2D descriptor-gen
        # runs in parallel with the sync-queue transposes); loads are
        # ordered by first use (w_p/w_s -> W_aff heads -> w_fp/w_fs).
        weng = nc.sync if xbar_io else nc.gpsimd
        w_p_sb = wpool.tile([P, NT, INNER], dt1, name="w_p_sb", tag="w_p_sb")
        weng.dma_start(w_p_sb[:], w_p.rearrange("(eo p) d -> p eo d", p=P))
        w_s_sb = wpool.tile([P, NT, INNER], dt1, name="w_s_sb", tag="w_s_sb")
        weng.dma_start(w_s_sb[:], w_s.rearrange("(eo p) d -> p eo d", p=P))
        w_aff_all = None
        if mybir.dt.size(dt1) <= 2:
            # bf16 W_aff fits resident in SBUF (4.2 MB): load once per core.
            w_aff_all = wpool.tile(
                [P, H, NT, D], dt1, name="w_aff_all", tag="w_aff_all"
            )
            for h in range(H):
                weng.dma_start(
                    w_aff_all[:, h], w_aff[h].rearrange("(eo p) f -> p eo f", p=P)
                )
        NCT = CONCAT // P  # 5
        w_fp_sb = wpool.tile([P, NCT, D], dt1, name="w_fp_sb", tag="w_fp_sb")
        weng.dma_start(w_fp_sb[:], w_fp.rearrange("(co p) n -> p co n", p=P))
        w_fs_sb = wpool.tile([P, NCT, D], dt1, name="w_fs_sb", tag="w_fs_sb")
        weng.dma_start(w_fs_sb[:], w_fs.rearrange("(co p) n -> p co n", p=P))

        # Identities AFTER the weight dma_starts: gpsimd runs memset/
        # affine_select serially before its SWDGE triggers, so creating them
        # first would delay weight loads issued from gpsimd.
        ident2 = ident_for(dt2)

        if not bias_trivial:
            bias_p_bc = consts.tile([P, D], F32, name="bias_p_bc", tag="bias_p_bc")
            nc.sync.dma_start(bias_p_bc[:], b_fp.partition_broadcast(P))
            bias_s_bc = consts.tile([P, D], F32, name="bias_s_bc", tag="bias_s_bc")
            nc.sync.dma_start(bias_s_bc[:], b_fs.partition_broadcast(P))

        def transpose_512(dst, src, idn, psdt):
            """dst[:, c, r*P:(r+1)*P] = src[:, r, c*P:(c+1)*P]^T for all r, c."""
            for c in range(NT):
                ps_t = psum.tile([P, L], psdt, name="ps_t", tag="tr")
                for r in range(NT):
                    nc.tensor.transpose(
                        ps_t[:, r * P:(r + 1) * P],
                        src[:, r, c * P:(c + 1) * P],
                        idn,
                    )
                nc.vector.tensor_copy(out=dst[:, c, :], in_=ps_t[:])

        for b in range(BPC):
            # ---- load + transpose P, S ----------------------------------
            pt, st = make_ptst(b)  # (e_in, eo, i) / (f_in, fo, j)

            if not masks_trivial:
                pm_sb = consts.tile([P, NT], F32, name="pm_sb", tag="pm_sb", bufs=2)
                sm_sb = consts.tile([P, NT], F32, name="sm_sb", tag="sm_sb", bufs=2)
                with nc.allow_non_contiguous_dma(reason="tiny 2KB mask load"):
                    nc.sync.dma_start(pm_sb[:], pmask[b].rearrange("(io p) -> p io", p=P))
                    nc.sync.dma_start(sm_sb[:], smask[b].rearrange("(jo p) -> p jo", p=P))
                pm_bc = consts.tile([P, L], F32, name="pm_bc", tag="pm_bc", bufs=2)
                nc.sync.dma_start(pm_bc[:], pmask[b].partition_broadcast(P))
                sm_bc = consts.tile([P, L], F32, name="sm_bc", tag="sm_bc", bufs=2)
                nc.sync.dma_start(sm_bc[:], smask[b].partition_broadcast(P))

            # ---- projections pp (i,dd), ps (j,dd), mask-scaled ----------
            pp = bpool.tile([P, NT, INNER], dt2, name="pp", tag="pp")
            ps = bpool.tile([P, NT, INNER], dt2, name="ps", tag="ps")
            for dst, src_t, w_sb, m_sb in (
                (pp, pt, w_p_sb, "pm"),
                (ps, st, w_s_sb, "sm"),
            ):
                for io in range(NT):
                    for dh in range(2):
                        ps_mm = psum.tile([P, 512], F32, name="ps_proj", tag="mm", bufs=MMB)
                        for eo in range(NT):
                            nc.tensor.matmul(
                                ps_mm[:],
                                src_t[:, eo, io * P:(io + 1) * P],
                                w_sb[:, eo, dh * 512:(dh + 1) * 512],
                                start=(eo == 0),
                                stop=(eo == NT - 1),
                            )
                        dsl = dst[:, io, dh * 512:(dh + 1) * 512]
                        if masks_trivial:
                            nc.vector.tensor_copy(out=dsl, in_=ps_mm[:])
                        else:
                            msb = pm_sb if m_sb == "pm" else sm_sb
                            nc.vector.tensor_scalar_mul(
                                dsl, ps_mm[:], msb[:, io:io + 1]
                            )

            # ---- per-head affinity chain + pooled weighted sums ---------
            pool_p = bpool.tile([P, L], dt1, name="pool_p", tag="pool_p")  # (d, j)
            pool_s = bpool.tile([P, L], dt1, name="pool_s", tag="pool_s")  # (d, i)

            for h in range(H):
                if w_aff_all is not None:
                    w_aff_h = w_aff_all[:, h]
                else:
                    w_aff_h = waffp.tile(
                        [P, NT, D], dt1, name="w_aff_h", tag="w_aff_h"
                    )
                    nc.sync.dma_start(
                        w_aff_h[:], w_aff[h].rearrange("(eo p) f -> p eo f", p=P)
                    )

                # PW^T (f, i) = W_aff[h]^T-contraction with P^T
                pwt = hpool.tile([P, NT, L], dt1, name="pwt", tag="pwt")
                for fo in range(NT):
                    ps_mm = psum.tile([P, 512], F32, name="ps_pw", tag="mm", bufs=MMB)
                    for eo in range(NT):
                        nc.tensor.matmul(
                            ps_mm[:],
                            w_aff_h[:, eo, fo * P:(fo + 1) * P],
                            pt[:, eo, :],
                            start=(eo == 0),
                            stop=(eo == NT - 1),
                        )
                    nc.scalar.copy(out=pwt[:, fo, :], in_=ps_mm[:])

                # A (i, j) = tanh(PW^T.T @ S^T)
                a_sb = hpool.tile([P, NT, L], dt2, name="a_sb", tag="a_sb")
                for io in range(NT):
                    ps_mm = psum.tile([P, 512], F32, name="ps_a", tag="mm", bufs=MMB)
                    for fo in range(NT):
                        nc.tensor.matmul(
                            ps_mm[:],
                            pwt[:, fo, io * P:(io + 1) * P],
                            st[:, fo, :],
                            start=(fo == 0),
                            stop=(fo == NT - 1),
                        )
                    nc.scalar.activation(out=a_sb[:, io, :], in_=ps_mm[:], func=TANH)

                # A^T (j, i)
                at_sb = hpool.tile([P, NT, L], dt2, name="at_sb", tag="at_sb")
                transpose_512(at_sb, a_sb, ident2, dt2)

                # wp_h^T (d, j) = pp_h^T-contraction with A ; pool over heads
                ps_wp = psum.tile([P, L], F32, name="ps_wp", tag="mm", bufs=MMB)
                for io in range(NT):
                    nc.tensor.matmul(
                        ps_wp[:],
                        pp[:, io, h * HD:(h + 1) * HD],
                        a_sb[:, io, :],
                        start=(io == 0),
                        stop=(io == NT - 1),
                    )
                if h == 0:
                    nc.vector.tensor_scalar_max(pool_p[:], ps_wp[:], 0.0)
                else:
                    nc.vector.tensor_max(out=pool_p[:], in0=pool_p[:], in1=ps_wp[:])

                # ws_h^T (d, i) = ps_h^T-contraction with A^T ; pool over heads
                ps_ws = psum.tile([P, L], F32, name="ps_ws", tag="mm", bufs=MMB)
                for jo in range(NT):
                    nc.tensor.matmul(
                        ps_ws[:],
                        ps[:, jo, h * HD:(h + 1) * HD],
                        at_sb[:, jo, :],
                        start=(jo == 0),
                        stop=(jo == NT - 1),
                    )
                if h == 0:
                    nc.vector.tensor_scalar_max(pool_s[:], ps_ws[:], 0.0)
                else:
                    nc.vector.tensor_max(out=pool_s[:], in0=pool_s[:], in1=ps_ws[:])

            if not masks_trivial:
                # wp^T pools scale by sm_j (free dim j); ws^T pools by pm_i.
                nc.vector.tensor_mul(out=pool_p[:], in0=pool_p[:], in1=sm_bc[:])
                nc.vector.tensor_mul(out=pool_s[:], in0=pool_s[:], in1=pm_bc[:])

            # ---- fused outputs ------------------------------------------
            for name_o, dst_d, lhs_t, pool_t, w_o, bias_bc in (
                ("o_p", out_p, pt, pool_s, w_fp_sb, "p"),
                ("o_s", out_s, st, pool_p, w_fs_sb, "s"),
            ):
                for io in range(NT):
                    ps_mm = psum.tile([P, 512], F32, name="ps_out", tag="mm", bufs=MMB)
                    for co in range(NT):
                        nc.tensor.matmul(
                            ps_mm[:],
                            lhs_t[:, co, io * P:(io + 1) * P],
                            w_o[:, co, :],
                            start=(co == 0),
                            stop=False,
                        )
                    nc.tensor.matmul(
                        ps_mm[:],
                        pool_t[:, io * P:(io + 1) * P],
                        w_o[:, NT, :],
                        start=False,
                        stop=True,
                    )
                    o_sb = iopool.tile(
                        [P, D], F32, name=name_o, tag=name_o, bufs=3
                    )
                    if bias_trivial:
                        nc.scalar.activation(out=o_sb[:], in_=ps_mm[:], func=RELU)
                    else:
                        bb = bias_p_bc if bias_bc == "p" else bias_s_bc
                        nc.vector.tensor_add(out=o_sb[:], in0=ps_mm[:], in1=bb[:])
                        nc.vector.tensor_scalar_max(o_sb[:], o_sb[:], 0.0)
                    nc.scalar.dma_start(
                        dst_d[b, io * P:(io + 1) * P, :], o_sb[:]
                    )


_PROGRAM_CACHE = {}


def _get_program(masks_trivial, bias_trivial):
    key = (masks_trivial, bias_trivial, PREC)
    if key not in _PROGRAM_CACHE:
        _PROGRAM_CACHE[key] = _build_program(masks_trivial, bias_trivial, PREC)
    return _PROGRAM_CACHE[key]


def kernel(
    primary, secondary, primary_mask, secondary_mask,
    W_aff, W_p, W_s, W_fp, b_fp, W_fs, b_fs,
    _trace=False,
):
    import ml_dtypes

    f32 = np.float32
    dt_io = ml_dtypes.bfloat16 if PREC == "b" else f32
    primary = np.ascontiguousarray(np.asarray(primary, f32).astype(dt_io))
    secondary = np.ascontiguousarray(np.asarray(secondary, f32).astype(dt_io))
    primary_mask = np.ascontiguousarray(np.asarray(primary_mask, f32))
    secondary_mask = np.ascontiguousarray(np.asarray(secondary_mask, f32))
    weights = {
        "W_aff": np.ascontiguousarray(np.asarray(W_aff, f32).astype(dt_io)),
        "W_p": np.ascontiguousarray(np.asarray(W_p, f32).astype(dt_io)),
        "W_s": np.ascontiguousarray(np.asarray(W_s, f32).astype(dt_io)),
        "W_fp": np.ascontiguousarray(np.asarray(W_fp, f32).astype(dt_io)),
        "b_fp": np.ascontiguousarray(np.asarray(b_fp, f32)),
        "W_fs": np.ascontiguousarray(np.asarray(W_fs, f32).astype(dt_io)),
        "b_fs": np.ascontiguousarray(np.asarray(b_fs, f32)),
    }

    masks_trivial = bool(
        (primary_mask == 1.0).all() and (secondary_mask == 1.0).all()
    )
    bias_trivial = not (weights["b_fp"].any() or weights["b_fs"].any())

    nc = _get_program(masks_trivial, bias_trivial)

    in_maps = []
    for c in range(NCORES):
        sl = slice(c * BPC, (c + 1) * BPC)
        in_maps.append(
            {
                "primary": primary[sl],
                "secondary": secondary[sl],
                "primary_mask": primary_mask[sl],
                "secondary_mask": secondary_mask[sl],
                **weights,
            }
        )

    res = bass_utils.run_bass_kernel_spmd(
        nc, in_maps, core_ids=list(range(NCORES)), trace=_trace
    )
    out_p = np.concatenate([r["out_p"] for r in res.results], axis=0)
    out_s = np.concatenate([r["out_s"] for r in res.results], axis=0)
    if _trace:
        kernel.last_results = res
    return out_p, out_s


# revision 34
# speedup vs baseline: 1.1104x; 1.1104x over previous
"""CoAttentionFusion Trainium2 kernel.

Full-input contract: kernel(**inputs) takes the complete (unsharded) numpy
inputs and returns (out_p, out_s) matching the fp32 reference. Internally
shards batch 16 -> 2 per core across 8 NeuronCores (weights replicated),
builds one SPMD Bass program, and runs it via run_bass_kernel_spmd.

Math per batch b (L1=L2=512, D=512, H=8, HD=128):
  aff_h = tanh(P @ W_aff[h] @ S^T) * (pm_i * sm_j)
  pp = (P @ W_p) head-split; ps = (S @ W_s) head-split
  wp_h = relu(aff_h^T @ pp_h)  -> pool_p = max_h wp_h   (B, L2, HD)
  ws_h = relu(aff_h  @ ps_h)  -> pool_s = max_h ws_h   (B, L1, HD)
  out_p = relu([P, pool_s] @ W_fp + b_fp)
  out_s = relu([S, pool_p] @ W_fs + b_fs)

Layout trick: every matmul contracting P over D needs P^T (D on partitions),
and the final fused matmuls need [P^T; pool_s^T] as lhsT, so we compute the
weighted sums directly in transposed orientation:
  ws_h^T (d,i) = ps_h^T-as-lhsT @ A^T ; wp_h^T (d,j) = pp_h-as-lhsT @ A
which makes the head pools land exactly in the lhsT layout of the final
matmuls. Masks (>=0) commute with relu/max and are folded into pp/ps rows
and one elementwise multiply on each pool.

Precision (K_PREC): "f32" all-fp32; "r" fp32r everywhere (PE runs fp32r at
4x the fp32 rate for 512-wide moving operands); "rb" fp32r for the affinity
chain + projections + final matmuls, bf16 for the post-tanh tensors.
"""

import os

import numpy as np

import concourse.bacc as bacc
import concourse.mybir as mybir
import concourse.tile as tile
from concourse import bass_utils
from concourse.masks import make_identity

# Problem constants (hardcoded per contract).
B = 16
L = 512  # L1 == L2
D = 512
H = 8
INNER = 1024
HD = INNER // H  # 128
CONCAT = D + HD  # 640
P = 128
NT = L // P  # 4
NCORES = 8
BPC = B // NCORES  # batches per core

F32 = mybir.dt.float32
F32R = mybir.dt.float32r
BF16 = mybir.dt.bfloat16

PREC = os.environ.get("K_PREC", "b")


def _build_program(masks_trivial: bool, bias_trivial: bool, prec: str):
    if prec == "f32":
        dt1, dt2, dt_in = F32, F32, F32
    elif prec == "r":
        dt1, dt2, dt_in = F32R, F32R, F32
    elif prec == "rb":
        dt1, dt2, dt_in = F32R, BF16, F32
    elif prec == "b":
        dt1, dt2, dt_in = BF16, BF16, BF16
    else:
        raise ValueError(prec)

    nc = bacc.Bacc(
        "TRN2",
        target_bir_lowering=False,
        debug=False,
        enable_asserts=False,
        num_devices=NCORES,
    )

    def din(name, shape, dt=F32):
        return nc.dram_tensor(name, list(shape), dt, kind="ExternalInput").ap()

    def dout(name, shape):
        return nc.dram_tensor(name, list(shape), F32, kind="ExternalOutput").ap()

    primary = din("primary", (BPC, L, D), dt_in)
    secondary = din("secondary", (BPC, L, D), dt_in)
    pmask = din("primary_mask", (BPC, L))
    smask = din("secondary_mask", (BPC, L))
    w_aff = din("W_aff", (H, D, D), dt1)
    w_p = din("W_p", (D, INNER), dt1)
    w_s = din("W_s", (D, INNER), dt1)
    w_fp = din("W_fp", (CONCAT, D), dt1)
    b_fp = din("b_fp", (D,))
    w_fs = din("W_fs", (CONCAT, D), dt1)
    b_fs = din("b_fs", (D,))
    out_p = dout("out_p", (BPC, L, D))
    out_s = dout("out_s", (BPC, L, D))

    with tile.TileContext(nc) as tc:
        _body(
            tc,
            primary, secondary, pmask, smask,
            w_aff, w_p, w_s, w_fp, b_fp, w_fs, b_fs,
            out_p, out_s,
            masks_trivial, bias_trivial, dt1, dt2, dt_in,
        )
    nc.compile()
    return nc


def _body(
    tc,
    primary, secondary, pmask, smask,
    w_aff, w_p, w_s, w_fp, b_fp, w_fs, b_fs,
    out_p, out_s,
    masks_trivial, bias_trivial, dt1, dt2, dt_in,
):
    nc = tc.nc
    TANH = mybir.ActivationFunctionType.Tanh
    RELU = mybir.ActivationFunctionType.Relu
    hbufs = 3 if dt2 == BF16 else 1

    with (
        tc.tile_pool(name="consts", bufs=1) as consts,
        tc.tile_pool(name="wpool", bufs=1) as wpool,
        tc.tile_pool(name="waffp", bufs=2) as waffp,
        tc.tile_pool(name="bpool", bufs=2 if dt1 == BF16 else 1) as bpool,
        tc.tile_pool(name="hpool", bufs=hbufs) as hpool,
        tc.tile_pool(name="iopool", bufs=2) as iopool,
        tc.tile_pool(name="adram", bufs=3, space="DRAM") as adram,
        tc.tile_pool(name="psum", bufs=2, space="PSUM") as psum,
    ):
        MMB = 5  # psum bufs for matmul accumulation groups

        idents = {}

        def ident_for(dt):
            if dt not in idents:
                name = f"ident_{dt.name}"
                t = consts.tile([P, P], dt, name=name, tag=name)
                if dt == F32R:
                    fi = ident_for(F32)
                    nc.vector.tensor_copy(out=t[:], in_=fi[:])
                else:
                    make_identity(nc, t)
                idents[dt] = t
            return idents[dt]

        xbar_io = dt1 == dt_in and mybir.dt.size(dt_in) == 2

        # pt/st per batch, memoized so batch 0's can be issued before the
        # weight burst (the XBAR transpose is a single shared resource and
        # crawls if it overlaps the 8.5 MB weight stream on the HBM).
        ptst = {}

        def make_ptst(b):
            if b not in ptst:
                pt = bpool.tile([P, NT, L], dt1, name="pt", tag="pt")
                st = bpool.tile([P, NT, L], dt1, name="st", tag="st")
                if xbar_io:
                    # pt[p, eo, i] = primary[b][i, eo*128+p]
                    nc.sync.dma_start_transpose(pt[:], primary[b])
                    nc.sync.dma_start_transpose(st[:], secondary[b])
                else:
                    p_nat = iopool.tile([P, NT, D], dt_in, name="p_nat", tag="p_nat")
                    nc.sync.dma_start(
                        p_nat[:], primary[b].rearrange("(io p) e -> p io e", p=P)
                    )
                    s_nat = iopool.tile([P, NT, D], dt_in, name="s_nat", tag="s_nat")
                    nc.sync.dma_start(
                        s_nat[:], secondary[b].rearrange("(jo p) e -> p jo e", p=P)
                    )
                    transpose_512(pt, p_nat, ident_for(dt_in), dt_in)
                    transpose_512(st, s_nat, ident_for(dt_in), dt_in)
                ptst[b] = (pt, st)
            return ptst[b]

        if xbar_io:
            make_ptst(0)

        # ---- resident weights -------------------------------------------
        # Weights ride the gpsimd sequencer (its DIRECT2D descriptor-gen
        # runs in parallel with the sync-queue transposes); loads are
        # ordered by first use (w_p/w_s -> W_aff heads -> w_fp/w_fs).
        weng = nc.sync if xbar_io else nc.gpsimd
        w_p_sb = wpool.tile([P, NT, INNER], dt1, name="w_p_sb", tag="w_p_sb")
        weng.dma_start(w_p_sb[:], w_p.rearrange("(eo p) d -> p eo d", p=P))
        w_s_sb = wpool.tile([P, NT, INNER], dt1, name="w_s_sb", tag="w_s_sb")
        weng.dma_start(w_s_sb[:], w_s.rearrange("(eo p) d -> p eo d", p=P))
        w_aff_all = None
        if mybir.dt.size(dt1) <= 2:
            # bf16 W_aff fits resident in SBUF (4.2 MB): load once per core.
            w_aff_all = wpool.tile(
                [P, H, NT, D], dt1, name="w_aff_all", tag="w_aff_all"
            )
            for h in range(H):
                weng.dma_start(
                    w_aff_all[:, h], w_aff[h].rearrange("(eo p) f -> p eo f", p=P)
                )
        NCT = CONCAT // P  # 5
        w_fp_sb = wpool.tile([P, NCT, D], dt1, name="w_fp_sb", tag="w_fp_sb")
        weng.dma_start(w_fp_sb[:], w_fp.rearrange("(co p) n -> p co n", p=P))
        w_fs_sb = wpool.tile([P, NCT, D], dt1, name="w_fs_sb", tag="w_fs_sb")
        weng.dma_start(w_fs_sb[:], w_fs.rearrange("(co p) n -> p co n", p=P))

        # Identities AFTER the weight dma_starts: gpsimd runs memset/
        # affine_select serially before its SWDGE triggers, so creating them
        # first would delay weight loads issued from gpsimd.
        ident2 = ident_for(dt2)

        if not bias_trivial:
            bias_p_bc = consts.tile([P, D], F32, name="bias_p_bc", tag="bias_p_bc")
            nc.sync.dma_start(bias_p_bc[:], b_fp.partition_broadcast(P))
            bias_s_bc = consts.tile([P, D], F32, name="bias_s_bc", tag="bias_s_bc")
            nc.sync.dma_start(bias_s_bc[:], b_fs.partition_broadcast(P))

        def transpose_512(dst, src, idn, psdt):
            """dst[:, c, r*P:(r+1)*P] = src[:, r, c*P:(c+1)*P]^T for all r, c."""
            for c in range(NT):
                ps_t = psum.tile([P, L], psdt, name="ps_t", tag="tr", bufs=3)
                for r in range(NT):
                    nc.tensor.transpose(
                        ps_t[:, r * P:(r + 1) * P],
                        src[:, r, c * P:(c + 1) * P],
                        idn,
                    )
                nc.vector.tensor_copy(out=dst[:, c, :], in_=ps_t[:])

        for b in range(BPC):
            # ---- load + transpose P, S ----------------------------------
            pt, st = make_ptst(b)  # (e_in, eo, i) / (f_in, fo, j)

            if not masks_trivial:
                pm_sb = consts.tile([P, NT], F32, name="pm_sb", tag="pm_sb", bufs=2)
                sm_sb = consts.tile([P, NT], F32, name="sm_sb", tag="sm_sb", bufs=2)
                with nc.allow_non_contiguous_dma(reason="tiny 2KB mask load"):
                    nc.sync.dma_start(pm_sb[:], pmask[b].rearrange("(io p) -> p io", p=P))
                    nc.sync.dma_start(sm_sb[:], smask[b].rearrange("(jo p) -> p jo", p=P))
                pm_bc = consts.tile([P, L], F32, name="pm_bc", tag="pm_bc", bufs=2)
                nc.sync.dma_start(pm_bc[:], pmask[b].partition_broadcast(P))
                sm_bc = consts.tile([P, L], F32, name="sm_bc", tag="sm_bc", bufs=2)
                nc.sync.dma_start(sm_bc[:], smask[b].partition_broadcast(P))

            # ---- projections pp (i,dd), ps (j,dd), mask-scaled ----------
            pp = bpool.tile([P, NT, INNER], dt2, name="pp", tag="pp")
            ps = bpool.tile([P, NT, INNER], dt2, name="ps", tag="ps")
            for dst, src_t, w_sb, m_sb in (
                (pp, pt, w_p_sb, "pm"),
                (ps, st, w_s_sb, "sm"),
            ):
                for io in range(NT):
                    for dh in range(2):
                        ps_mm = psum.tile([P, 512], F32, name="ps_proj", tag="mm", bufs=MMB)
                        for eo in range(NT):
                            nc.tensor.matmul(
                                ps_mm[:],
                                src_t[:, eo, io * P:(io + 1) * P],
                                w_sb[:, eo, dh * 512:(dh + 1) * 512],
                                start=(eo == 0),
                                stop=(eo == NT - 1),
                            )
                        dsl = dst[:, io, dh * 512:(dh + 1) * 512]
                        if masks_trivial:
                            nc.vector.tensor_copy(out=dsl, in_=ps_mm[:])
                        else:
                            msb = pm_sb if m_sb == "pm" else sm_sb
                            nc.vector.tensor_scalar_mul(
                                dsl, ps_mm[:], msb[:, io:io + 1]
                            )

            # ---- per-head affinity chain + pooled weighted sums ---------
            pool_p = bpool.tile([P, L], dt1, name="pool_p", tag="pool_p")  # (d, j)
            pool_s = bpool.tile([P, L], dt1, name="pool_s", tag="pool_s")  # (d, i)

            for h in range(H):
                if w_aff_all is not None:
                    w_aff_h = w_aff_all[:, h]
                else:
                    w_aff_h = waffp.tile(
                        [P, NT, D], dt1, name="w_aff_h", tag="w_aff_h"
                    )
                    nc.sync.dma_start(
                        w_aff_h[:], w_aff[h].rearrange("(eo p) f -> p eo f", p=P)
                    )

                # PW^T (f, i) = W_aff[h]^T-contraction with P^T
                pwt = hpool.tile([P, NT, L], dt1, name="pwt", tag="pwt")
                for fo in range(NT):
                    ps_mm = psum.tile([P, 512], F32, name="ps_pw", tag="mm", bufs=MMB)
                    for eo in range(NT):
                        nc.tensor.matmul(
                            ps_mm[:],
                            w_aff_h[:, eo, fo * P:(fo + 1) * P],
                            pt[:, eo, :],
                            start=(eo == 0),
                            stop=(eo == NT - 1),
                        )
                    nc.scalar.copy(out=pwt[:, fo, :], in_=ps_mm[:])

                # A (i, j) = tanh(PW^T.T @ S^T)
                a_sb = hpool.tile([P, NT, L], dt2, name="a_sb", tag="a_sb")
                for io in range(NT):
                    ps_mm = psum.tile([P, 512], F32, name="ps_a", tag="mm", bufs=MMB)
                    for fo in range(NT):
                        nc.tensor.matmul(
                            ps_mm[:],
                            pwt[:, fo, io * P:(io + 1) * P],
                            st[:, fo, :],
                            start=(fo == 0),
                            stop=(fo == NT - 1),
                        )
                    nc.scalar.activation(out=a_sb[:, io, :], in_=ps_mm[:], func=TANH)

                # A^T (j, i)
                at_sb = hpool.tile([P, NT, L], dt2, name="at_sb", tag="at_sb")
                transpose_512(at_sb, a_sb, ident2, dt2)

                # wp_h^T (d, j) = pp_h^T-contraction with A ; pool over heads
                ps_wp = psum.tile([P, L], F32, name="ps_wp", tag="mm", bufs=MMB)
                for io in range(NT):
                    nc.tensor.matmul(
                        ps_wp[:],
                        pp[:, io, h * HD:(h + 1) * HD],
                        a_sb[:, io, :],
                        start=(io == 0),
                        stop=(io == NT - 1),
                    )
                if h == 0:
                    nc.vector.tensor_scalar_max(pool_p[:], ps_wp[:], 0.0)
                else:
                    nc.vector.tensor_max(out=pool_p[:], in0=pool_p[:], in1=ps_wp[:])

                # ws_h^T (d, i) = ps_h^T-contraction with A^T ; pool over heads
                ps_ws = psum.tile([P, L], F32, name="ps_ws", tag="mm", bufs=MMB)
                for jo in range(NT):
                    nc.tensor.matmul(
                        ps_ws[:],
                        ps[:, jo, h * HD:(h + 1) * HD],
                        at_sb[:, jo, :],
                        start=(jo == 0),
                        stop=(jo == NT - 1),
                    )
                if h == 0:
                    nc.vector.tensor_scalar_max(pool_s[:], ps_ws[:], 0.0)
                else:
                    nc.vector.tensor_max(out=pool_s[:], in0=pool_s[:], in1=ps_ws[:])

            if not masks_trivial:
                # wp^T pools scale by sm_j (free dim j); ws^T pools by pm_i.
                nc.vector.tensor_mul(out=pool_p[:], in0=pool_p[:], in1=sm_bc[:])
                nc.vector.tensor_mul(out=pool_s[:], in0=pool_s[:], in1=pm_bc[:])

            # ---- fused outputs ------------------------------------------
            for name_o, dst_d, lhs_t, pool_t, w_o, bias_bc in (
                ("o_p", out_p, pt, pool_s, w_fp_sb, "p"),
                ("o_s", out_s, st, pool_p, w_fs_sb, "s"),
            ):
                for io in range(NT):
                    ps_mm = psum.tile([P, 512], F32, name="ps_out", tag="mm", bufs=MMB)
                    for co in range(NT):
                        nc.tensor.matmul(
                            ps_mm[:],
                            lhs_t[:, co, io * P:(io + 1) * P],
                            w_o[:, co, :],
                            start=(co == 0),
                            stop=False,
                        )
                    nc.tensor.matmul(
                        ps_mm[:],
                        pool_t[:, io * P:(io + 1) * P],
                        w_o[:, NT, :],
                        start=False,
                        stop=True,
                    )
                    o_sb = iopool.tile(
                        [P, D], F32, name=name_o, tag=name_o, bufs=3
                    )
                    if bias_trivial:
                        nc.scalar.activation(out=o_sb[:], in_=ps_mm[:], func=RELU)
                    else:
                        bb = bias_p_bc if bias_bc == "p" else bias_s_bc
                        nc.vector.tensor_add(out=o_sb[:], in0=ps_mm[:], in1=bb[:])
                        nc.vector.tensor_scalar_max(o_sb[:], o_sb[:], 0.0)
                    nc.scalar.dma_start(
                        dst_d[b, io * P:(io + 1) * P, :], o_sb[:]
                    )


_PROGRAM_CACHE = {}


def _get_program(masks_trivial, bias_trivial):
    key = (masks_trivial, bias_trivial, PREC)
    if key not in _PROGRAM_CACHE:
        _PROGRAM_CACHE[key] = _build_program(masks_trivial, bias_trivial, PREC)
    return _PROGRAM_CACHE[key]


def kernel(
    primary, secondary, primary_mask, secondary_mask,
    W_aff, W_p, W_s, W_fp, b_fp, W_fs, b_fs,
    _trace=False,
):
    import ml_dtypes

    f32 = np.float32
    dt_io = ml_dtypes.bfloat16 if PREC == "b" else f32
    primary = np.ascontiguousarray(np.asarray(primary, f32).astype(dt_io))
    secondary = np.ascontiguousarray(np.asarray(secondary, f32).astype(dt_io))
    primary_mask = np.ascontiguousarray(np.asarray(primary_mask, f32))
    secondary_mask = np.ascontiguousarray(np.asarray(secondary_mask, f32))
    weights = {
        "W_aff": np.ascontiguousarray(np.asarray(W_aff, f32).astype(dt_io)),
        "W_p": np.ascontiguousarray(np.asarray(W_p, f32).astype(dt_io)),
        "W_s": np.ascontiguousarray(np.asarray(W_s, f32).astype(dt_io)),
        "W_fp": np.ascontiguousarray(np.asarray(W_fp, f32).astype(dt_io)),
        "b_fp": np.ascontiguousarray(np.asarray(b_fp, f32)),
        "W_fs": np.ascontiguousarray(np.asarray(W_fs, f32).astype(dt_io)),
        "b_fs": np.ascontiguousarray(np.asarray(b_fs, f32)),
    }

    masks_trivial = bool(
        (primary_mask == 1.0).all() and (secondary_mask == 1.0).all()
    )
    bias_trivial = not (weights["b_fp"].any() or weights["b_fs"].any())

    nc = _get_program(masks_trivial, bias_trivial)

    in_maps = []
    for c in range(NCORES):
        sl = slice(c * BPC, (c + 1) * BPC)
        in_maps.append(
            {
                "primary": primary[sl],
                "secondary": secondary[sl],
                "primary_mask": primary_mask[sl],
                "secondary_mask": secondary_mask[sl],
                **weights,
            }
        )

    res = bass_utils.run_bass_kernel_spmd(
        nc, in_maps, core_ids=list(range(NCORES)), trace=_trace
    )
    out_p = np.concatenate([r["out_p"] for r in res.results], axis=0)
    out_s = np.concatenate([r["out_s"] for r in res.results], axis=0)
    if _trace:
        kernel.last_results = res
    return out_p, out_s


# revision 38
# speedup vs baseline: 1.1126x; 1.0019x over previous
"""CoAttentionFusion Trainium2 kernel.

Full-input contract: kernel(**inputs) takes the complete (unsharded) numpy
inputs and returns (out_p, out_s) matching the fp32 reference. Internally
shards batch 16 -> 2 per core across 8 NeuronCores (weights replicated),
builds one SPMD Bass program, and runs it via run_bass_kernel_spmd.

Math per batch b (L1=L2=512, D=512, H=8, HD=128):
  aff_h = tanh(P @ W_aff[h] @ S^T) * (pm_i * sm_j)
  pp = (P @ W_p) head-split; ps = (S @ W_s) head-split
  wp_h = relu(aff_h^T @ pp_h)  -> pool_p = max_h wp_h   (B, L2, HD)
  ws_h = relu(aff_h  @ ps_h)  -> pool_s = max_h ws_h   (B, L1, HD)
  out_p = relu([P, pool_s] @ W_fp + b_fp)
  out_s = relu([S, pool_p] @ W_fs + b_fs)

Layout trick: every matmul contracting P over D needs P^T (D on partitions),
and the final fused matmuls need [P^T; pool_s^T] as lhsT, so we compute the
weighted sums directly in transposed orientation:
  ws_h^T (d,i) = ps_h^T-as-lhsT @ A^T ; wp_h^T (d,j) = pp_h-as-lhsT @ A
which makes the head pools land exactly in the lhsT layout of the final
matmuls. Masks (>=0) commute with relu/max and are folded into pp/ps rows
and one elementwise multiply on each pool.

Precision (K_PREC): "f32" all-fp32; "r" fp32r everywhere (PE runs fp32r at
4x the fp32 rate for 512-wide moving operands); "rb" fp32r for the affinity
chain + projections + final matmuls, bf16 for the post-tanh tensors.
"""

import os

import numpy as np

import concourse.bacc as bacc
import concourse.mybir as mybir
import concourse.tile as tile
from concourse import bass_utils
from concourse.masks import make_identity

# Problem constants (hardcoded per contract).
B = 16
L = 512  # L1 == L2
D = 512
H = 8
INNER = 1024
HD = INNER // H  # 128
CONCAT = D + HD  # 640
P = 128
NT = L // P  # 4
NCORES = 8
BPC = B // NCORES  # batches per core

F32 = mybir.dt.float32
F32R = mybir.dt.float32r
BF16 = mybir.dt.bfloat16

PREC = os.environ.get("K_PREC", "b")


def _build_program(masks_trivial: bool, bias_trivial: bool, prec: str):
    if prec == "f32":
        dt1, dt2, dt_in = F32, F32, F32
    elif prec == "r":
        dt1, dt2, dt_in = F32R, F32R, F32
    elif prec == "rb":
        dt1, dt2, dt_in = F32R, BF16, F32
    elif prec == "b":
        dt1, dt2, dt_in = BF16, BF16, BF16
    else:
        raise ValueError(prec)

    nc = bacc.Bacc(
        "TRN2",
        target_bir_lowering=False,
        debug=False,
        enable_asserts=False,
        num_devices=NCORES,
    )

    def din(name, shape, dt=F32):
        return nc.dram_tensor(name, list(shape), dt, kind="ExternalInput").ap()

    def dout(name, shape):
        return nc.dram_tensor(name, list(shape), F32, kind="ExternalOutput").ap()

    primary = din("primary", (BPC, L, D), dt_in)
    secondary = din("secondary", (BPC, L, D), dt_in)
    pmask = din("primary_mask", (BPC, L))
    smask = din("secondary_mask", (BPC, L))
    w_aff = din("W_aff", (H, D, D), dt1)
    w_p = din("W_p", (D, INNER), dt1)
    w_s = din("W_s", (D, INNER), dt1)
    w_fp = din("W_fp", (CONCAT, D), dt1)
    b_fp = din("b_fp", (D,))
    w_fs = din("W_fs", (CONCAT, D), dt1)
    b_fs = din("b_fs", (D,))
    out_p = dout("out_p", (BPC, L, D))
    out_s = dout("out_s", (BPC, L, D))

    with tile.TileContext(nc) as tc:
        _body(
            tc,
            primary, secondary, pmask, smask,
            w_aff, w_p, w_s, w_fp, b_fp, w_fs, b_fs,
            out_p, out_s,
            masks_trivial, bias_trivial, dt1, dt2, dt_in,
        )
    nc.compile()
    return nc


def _body(
    tc,
    primary, secondary, pmask, smask,
    w_aff, w_p, w_s, w_fp, b_fp, w_fs, b_fs,
    out_p, out_s,
    masks_trivial, bias_trivial, dt1, dt2, dt_in,
):
    nc = tc.nc
    TANH = mybir.ActivationFunctionType.Tanh
    RELU = mybir.ActivationFunctionType.Relu
    hbufs = 3 if dt1 == BF16 else (2 if dt2 == BF16 else 1)

    with (
        tc.tile_pool(name="consts", bufs=1) as consts,
        tc.tile_pool(name="wpool", bufs=1) as wpool,
        tc.tile_pool(name="waffp", bufs=2) as waffp,
        tc.tile_pool(name="bpool", bufs=2 if dt1 == BF16 else 1) as bpool,
        tc.tile_pool(name="hpool", bufs=hbufs) as hpool,
        tc.tile_pool(name="iopool", bufs=2) as iopool,
        tc.tile_pool(name="adram", bufs=3, space="DRAM") as adram,
        tc.tile_pool(name="psum", bufs=2, space="PSUM") as psum,
    ):
        MMB = 5  # psum bufs for matmul accumulation groups

        idents = {}

        def ident_for(dt):
            if dt not in idents:
                name = f"ident_{dt.name}"
                t = consts.tile([P, P], dt, name=name, tag=name)
                if dt == F32R:
                    fi = ident_for(F32)
                    nc.vector.tensor_copy(out=t[:], in_=fi[:])
                else:
                    make_identity(nc, t)
                idents[dt] = t
            return idents[dt]

        xbar_io = dt1 == dt_in and mybir.dt.size(dt_in) == 2

        if xbar_io:
            # gpsimd is otherwise idle in this path: build the identity first
            # and issue PE warm-up transposes so the HAM clock-gate is at 8/8
            # (2.4 GHz) by the time the first data-dependent matmul issues
            # (~15us in, waiting on pt + w_p). They also soak the cold-clock
            # period that the first ~3.4us of real matmuls would otherwise pay.
            wident = ident_for(dt2)
            for _ in range(40):
                warm = psum.tile([P, P], dt2, name="warm", tag="tr", bufs=3)
                nc.tensor.transpose(warm[:], wident[:], wident[:])

        # pt/st per batch, memoized so batch 0's can be issued before the
        # weight burst (the XBAR transpose is a single shared resource and
        # crawls if it overlaps the 8.5 MB weight stream on the HBM).
        ptst = {}

        def make_ptst(b):
            if b not in ptst:
                pt = bpool.tile([P, NT, L], dt1, name="pt", tag="pt")
                st = bpool.tile([P, NT, L], dt1, name="st", tag="st")
                if xbar_io:
                    # pt[p, eo, i] = primary[b][i, eo*128+p]
                    nc.sync.dma_start_transpose(pt[:], primary[b])
                    nc.sync.dma_start_transpose(st[:], secondary[b])
                else:
                    p_nat = iopool.tile([P, NT, D], dt_in, name="p_nat", tag="p_nat")
                    nc.sync.dma_start(
                        p_nat[:], primary[b].rearrange("(io p) e -> p io e", p=P)
                    )
                    s_nat = iopool.tile([P, NT, D], dt_in, name="s_nat", tag="s_nat")
                    nc.sync.dma_start(
                        s_nat[:], secondary[b].rearrange("(jo p) e -> p jo e", p=P)
                    )
                    transpose_512(pt, p_nat, ident_for(dt_in), dt_in)
                    transpose_512(st, s_nat, ident_for(dt_in), dt_in)
                ptst[b] = (pt, st)
            return ptst[b]

        if xbar_io:
            make_ptst(0)

        # ---- resident weights -------------------------------------------
        # Weights ride the gpsimd sequencer (its DIRECT2D descriptor-gen
        # runs in parallel with the sync-queue transposes); loads are
        # ordered by first use (w_p/w_s -> W_aff heads -> w_fp/w_fs).
        weng = nc.sync if xbar_io else nc.gpsimd
        w_aff_all = None
        if mybir.dt.size(dt1) <= 2:
            # bf16 W_aff fits resident in SBUF (4.2 MB): load once per core.
            # Head 0's slice is loaded first: batch 0 runs its head-0
            # affinity stage before the projections, so this is the first
            # weight the PE waits on.
            w_aff_all = wpool.tile(
                [P, H, NT, D], dt1, name="w_aff_all", tag="w_aff_all"
            )
            weng.dma_start(
                w_aff_all[:, 0], w_aff[0].rearrange("(eo p) f -> p eo f", p=P)
            )
        w_p_sb = wpool.tile([P, NT, INNER], dt1, name="w_p_sb", tag="w_p_sb")
        weng.dma_start(w_p_sb[:], w_p.rearrange("(eo p) d -> p eo d", p=P))
        w_s_sb = wpool.tile([P, NT, INNER], dt1, name="w_s_sb", tag="w_s_sb")
        weng.dma_start(w_s_sb[:], w_s.rearrange("(eo p) d -> p eo d", p=P))
        if w_aff_all is not None:
            for h in range(1, H):
                weng.dma_start(
                    w_aff_all[:, h], w_aff[h].rearrange("(eo p) f -> p eo f", p=P)
                )
        NCT = CONCAT // P  # 5
        w_fp_sb = wpool.tile([P, NCT, D], dt1, name="w_fp_sb", tag="w_fp_sb")
        weng.dma_start(w_fp_sb[:], w_fp.rearrange("(co p) n -> p co n", p=P))
        w_fs_sb = wpool.tile([P, NCT, D], dt1, name="w_fs_sb", tag="w_fs_sb")
        weng.dma_start(w_fs_sb[:], w_fs.rearrange("(co p) n -> p co n", p=P))

        # Identities AFTER the weight dma_starts: gpsimd runs memset/
        # affine_select serially before its SWDGE triggers, so creating them
        # first would delay weight loads issued from gpsimd.
        ident2 = ident_for(dt2)

        if not bias_trivial:
            bias_p_bc = consts.tile([P, D], F32, name="bias_p_bc", tag="bias_p_bc")
            nc.sync.dma_start(bias_p_bc[:], b_fp.partition_broadcast(P))
            bias_s_bc = consts.tile([P, D], F32, name="bias_s_bc", tag="bias_s_bc")
            nc.sync.dma_start(bias_s_bc[:], b_fs.partition_broadcast(P))

        def transpose_512(dst, src, idn, psdt):
            """dst[:, c, r*P:(r+1)*P] = src[:, r, c*P:(c+1)*P]^T for all r, c."""
            for c in range(NT):
                ps_t = psum.tile([P, L], psdt, name="ps_t", tag="tr", bufs=3)
                for r in range(NT):
                    nc.tensor.transpose(
                        ps_t[:, r * P:(r + 1) * P],
                        src[:, r, c * P:(c + 1) * P],
                        idn,
                    )
                nc.vector.tensor_copy(out=dst[:, c, :], in_=ps_t[:])

        for b in range(BPC):
            # ---- load + transpose P, S ----------------------------------
            pt, st = make_ptst(b)  # (e_in, eo, i) / (f_in, fo, j)

            if not masks_trivial:
                pm_sb = consts.tile([P, NT], F32, name="pm_sb", tag="pm_sb", bufs=2)
                sm_sb = consts.tile([P, NT], F32, name="sm_sb", tag="sm_sb", bufs=2)
                with nc.allow_non_contiguous_dma(reason="tiny 2KB mask load"):
                    nc.sync.dma_start(pm_sb[:], pmask[b].rearrange("(io p) -> p io", p=P))
                    nc.sync.dma_start(sm_sb[:], smask[b].rearrange("(jo p) -> p jo", p=P))
                pm_bc = consts.tile([P, L], F32, name="pm_bc", tag="pm_bc", bufs=2)
                nc.sync.dma_start(pm_bc[:], pmask[b].partition_broadcast(P))
                sm_bc = consts.tile([P, L], F32, name="sm_bc", tag="sm_bc", bufs=2)
                nc.sync.dma_start(sm_bc[:], smask[b].partition_broadcast(P))

            # ---- projections pp (i,dd), ps (j,dd), mask-scaled ----------
            pp = bpool.tile([P, NT, INNER], dt2, name="pp", tag="pp")
            ps = bpool.tile([P, NT, INNER], dt2, name="ps", tag="ps")

            def proj_stage():
                for dst, src_t, w_sb, m_sb in (
                    (pp, pt, w_p_sb, "pm"),
                    (ps, st, w_s_sb, "sm"),
                ):
                    for io in range(NT):
                        for dh in range(2):
                            ps_mm = psum.tile(
                                [P, 512], F32, name="ps_proj", tag="mm", bufs=MMB
                            )
                            for eo in range(NT):
                                nc.tensor.matmul(
                                    ps_mm[:],
                                    src_t[:, eo, io * P:(io + 1) * P],
                                    w_sb[:, eo, dh * 512:(dh + 1) * 512],
                                    start=(eo == 0),
                                    stop=(eo == NT - 1),
                                )
                            dsl = dst[:, io, dh * 512:(dh + 1) * 512]
                            if masks_trivial:
                                nc.vector.tensor_copy(out=dsl, in_=ps_mm[:])
                            else:
                                msb = pm_sb if m_sb == "pm" else sm_sb
                                nc.vector.tensor_scalar_mul(
                                    dsl, ps_mm[:], msb[:, io:io + 1]
                                )

            # ---- per-head affinity chain + pooled weighted sums ---------
            pool_p = bpool.tile([P, L], dt1, name="pool_p", tag="pool_p")  # (d, j)
            pool_s = bpool.tile([P, L], dt1, name="pool_s", tag="pool_s")  # (d, i)

            def head_front(h):
                """PW^T -> A = tanh(.) -> A^T; needs only pt/st/W_aff[h]."""
                if w_aff_all is not None:
                    w_aff_h = w_aff_all[:, h]
                else:
                    w_aff_h = waffp.tile(
                        [P, NT, D], dt1, name="w_aff_h", tag="w_aff_h"
                    )
                    nc.sync.dma_start(
                        w_aff_h[:], w_aff[h].rearrange("(eo p) f -> p eo f", p=P)
                    )

                # PW^T (f, i) = W_aff[h]^T-contraction with P^T
                pwt = hpool.tile([P, NT, L], dt1, name="pwt", tag="pwt")
                for fo in range(NT):
                    ps_mm = psum.tile([P, 512], F32, name="ps_pw", tag="mm", bufs=MMB)
                    for eo in range(NT):
                        nc.tensor.matmul(
                            ps_mm[:],
                            w_aff_h[:, eo, fo * P:(fo + 1) * P],
                            pt[:, eo, :],
                            start=(eo == 0),
                            stop=(eo == NT - 1),
                        )
                    nc.scalar.copy(out=pwt[:, fo, :], in_=ps_mm[:])

                # A (i, j) = tanh(PW^T.T @ S^T)
                a_sb = hpool.tile([P, NT, L], dt2, name="a_sb", tag="a_sb")
                for io in range(NT):
                    ps_mm = psum.tile([P, 512], F32, name="ps_a", tag="mm", bufs=MMB)
                    for fo in range(NT):
                        nc.tensor.matmul(
                            ps_mm[:],
                            pwt[:, fo, io * P:(io + 1) * P],
                            st[:, fo, :],
                            start=(fo == 0),
                            stop=(fo == NT - 1),
                        )
                    nc.scalar.activation(out=a_sb[:, io, :], in_=ps_mm[:], func=TANH)

                # A^T (j, i)
                at_sb = hpool.tile([P, NT, L], dt2, name="at_sb", tag="at_sb")
                transpose_512(at_sb, a_sb, ident2, dt2)
                return a_sb, at_sb

            def head_back(h, a_sb, at_sb):
                """Pooled weighted sums; needs pp/ps."""
                # wp_h^T (d, j) = pp_h^T-contraction with A ; pool over heads
                ps_wp = psum.tile([P, L], F32, name="ps_wp", tag="mm", bufs=MMB)
                for io in range(NT):
                    nc.tensor.matmul(
                        ps_wp[:],
                        pp[:, io, h * HD:(h + 1) * HD],
                        a_sb[:, io, :],
                        start=(io == 0),
                        stop=(io == NT - 1),
                    )
                if h == 0:
                    nc.vector.tensor_scalar_max(pool_p[:], ps_wp[:], 0.0)
                else:
                    nc.vector.tensor_max(out=pool_p[:], in0=pool_p[:], in1=ps_wp[:])

                # ws_h^T (d, i) = ps_h^T-contraction with A^T ; pool over heads
                ps_ws = psum.tile([P, L], F32, name="ps_ws", tag="mm", bufs=MMB)
                for jo in range(NT):
                    nc.tensor.matmul(
                        ps_ws[:],
                        ps[:, jo, h * HD:(h + 1) * HD],
                        at_sb[:, jo, :],
                        start=(jo == 0),
                        stop=(jo == NT - 1),
                    )
                if h == 0:
                    nc.vector.tensor_scalar_max(pool_s[:], ps_ws[:], 0.0)
                else:
                    nc.vector.tensor_max(out=pool_s[:], in0=pool_s[:], in1=ps_ws[:])

            if b == 0 and w_aff_all is not None:
                # Head 0's affinity stage only needs pt + W_aff[0] (the first
                # arrivals) — run it before the projections so the PE starts
                # ~4us earlier.
                front0 = head_front(0)
                proj_stage()
                head_back(0, *front0)
                for h in range(1, H):
                    head_back(h, *head_front(h))
            else:
                proj_stage()
                for h in range(H):
                    head_back(h, *head_front(h))

            if not masks_trivial:
                # wp^T pools scale by sm_j (free dim j); ws^T pools by pm_i.
                nc.vector.tensor_mul(out=pool_p[:], in0=pool_p[:], in1=sm_bc[:])
                nc.vector.tensor_mul(out=pool_s[:], in0=pool_s[:], in1=pm_bc[:])

            # ---- fused outputs ------------------------------------------
            for name_o, dst_d, lhs_t, pool_t, w_o, bias_bc in (
                ("o_p", out_p, pt, pool_s, w_fp_sb, "p"),
                ("o_s", out_s, st, pool_p, w_fs_sb, "s"),
            ):
                for io in range(NT):
                    ps_mm = psum.tile([P, 512], F32, name="ps_out", tag="mm", bufs=MMB)
                    for co in range(NT):
                        nc.tensor.matmul(
                            ps_mm[:],
                            lhs_t[:, co, io * P:(io + 1) * P],
                            w_o[:, co, :],
                            start=(co == 0),
                            stop=False,
                        )
                    nc.tensor.matmul(
                        ps_mm[:],
                        pool_t[:, io * P:(io + 1) * P],
                        w_o[:, NT, :],
                        start=False,
                        stop=True,
                    )
                    o_sb = iopool.tile(
                        [P, D], F32, name=name_o, tag=name_o, bufs=3
                    )
                    if bias_trivial:
                        nc.scalar.activation(out=o_sb[:], in_=ps_mm[:], func=RELU)
                    else:
                        bb = bias_p_bc if bias_bc == "p" else bias_s_bc
                        nc.vector.tensor_add(out=o_sb[:], in0=ps_mm[:], in1=bb[:])
                        nc.vector.tensor_scalar_max(o_sb[:], o_sb[:], 0.0)
                    nc.scalar.dma_start(
                        dst_d[b, io * P:(io + 1) * P, :], o_sb[:]
                    )


_PROGRAM_CACHE = {}


def _get_program(masks_trivial, bias_trivial):
    key = (masks_trivial, bias_trivial, PREC)
    if key not in _PROGRAM_CACHE:
        _PROGRAM_CACHE[key] = _build_program(masks_trivial, bias_trivial, PREC)
    return _PROGRAM_CACHE[key]


def kernel(
    primary, secondary, primary_mask, secondary_mask,
    W_aff, W_p, W_s, W_fp, b_fp, W_fs, b_fs,
    _trace=False,
):
    import ml_dtypes

    f32 = np.float32
    dt_io = ml_dtypes.bfloat16 if PREC == "b" else f32
    primary = np.ascontiguousarray(np.asarray(primary, f32).astype(dt_io))
    secondary = np.ascontiguousarray(np.asarray(secondary, f32).astype(dt_io))
    primary_mask = np.ascontiguousarray(np.asarray(primary_mask, f32))
    secondary_mask = np.ascontiguousarray(np.asarray(secondary_mask, f32))
    weights = {
        "W_aff": np.ascontiguousarray(np.asarray(W_aff, f32).astype(dt_io)),
        "W_p": np.ascontiguousarray(np.asarray(W_p, f32).astype(dt_io)),
        "W_s": np.ascontiguousarray(np.asarray(W_s, f32).astype(dt_io)),
        "W_fp": np.ascontiguousarray(np.asarray(W_fp, f32).astype(dt_io)),
        "b_fp": np.ascontiguousarray(np.asarray(b_fp, f32)),
        "W_fs": np.ascontiguousarray(np.asarray(W_fs, f32).astype(dt_io)),
        "b_fs": np.ascontiguousarray(np.asarray(b_fs, f32)),
    }

    masks_trivial = bool(
        (primary_mask == 1.0).all() and (secondary_mask == 1.0).all()
    )
    bias_trivial = not (weights["b_fp"].any() or weights["b_fs"].any())

    nc = _get_program(masks_trivial, bias_trivial)

    in_maps = []
    for c in range(NCORES):
        sl = slice(c * BPC, (c + 1) * BPC)
        in_maps.append(
            {
                "primary": primary[sl],
                "secondary": secondary[sl],
                "primary_mask": primary_mask[sl],
                "secondary_mask": secondary_mask[sl],
                **weights,
            }
        )

    res = bass_utils.run_bass_kernel_spmd(
        nc, in_maps, core_ids=list(range(NCORES)), trace=_trace
    )
    out_p = np.concatenate([r["out_p"] for r in res.results], axis=0)
    out_s = np.concatenate([r["out_s"] for r in res.results], axis=0)
    if _trace:
        kernel.last_results = res
    return out_p, out_s
